# revision 1
# baseline (speedup 1.0000x reference)
"""MetaPathEncoder (4x GraphConv + mean fusion) as a Bass/Tile SPMD kernel on 8 TRN2 cores.

Strategy (1D dst-node sharding, all 4 metapaths per core):
  - Each core owns 1250 output rows (10000/8). Edges are bucketed on host by
    (core, path, 128-row dst tile); both GraphConv norms and the 1/4 mean are
    folded into a single per-edge scale c_e computed on host.
  - On device, per (tile, path): dma_gather the edge source rows (bf16) from
    HBM, build the scaled one-hot matrix S[e, dst_local] = c_e on DVE
    (iota == dstl fused with * c_e), and segment-sum via PE matmuls
    accumulating in fp32 PSUM: h[dst, :] = sum_b S_b.T @ X_b.
  - h is transposed on the PE (identity matmul) to get fi-on-partitions, then
    16 accumulating matmuls apply the four 512x512 weights: out = sum_p h_p @ W_p.
  - Bias mean is added and the [1250, 512] fp32 result is DMA'd out; the host
    concatenates the 8 shards.
"""
import sys

for _p in ("/opt/trn_rl_repo",):
    if _p not in sys.path:
        sys.path.insert(0, _p)

import numpy as np
import ml_dtypes

import concourse.bass as bass
import concourse.tile as tile
from concourse import bacc, mybir
from concourse.bass_utils import run_bass_kernel_spmd
BF16 = ml_dtypes.bfloat16

N_NODES = 10000
N_PATHS = 4
IN_DIM = 512
OUT_DIM = 512
NCORES = 8
ROWS_PER_CORE = N_NODES // NCORES  # 1250
NTILES = (ROWS_PER_CORE + 127) // 128  # 10 (last tile has 98 rows)
NCALLS = NTILES * N_PATHS  # 40 gather calls per core

_program_cache: dict[int, object] = {}


def _build_program(B: int):
    """Build the SPMD Bass program for B gather blocks per (tile, path)."""
    if B in _program_cache:
        return _program_cache[B]

    dt = mybir.dt
    nc = bacc.Bacc("TRN2", target_bir_lowering=False, debug=False, num_devices=NCORES)

    featd = nc.dram_tensor("feat", [N_NODES, IN_DIM], dt.bfloat16, kind="ExternalInput").ap()
    idxd = nc.dram_tensor("idx", [128, NCALLS * B * 8], dt.int16, kind="ExternalInput").ap()
    dstld = nc.dram_tensor("dstl", [128, NCALLS * B], dt.float32, kind="ExternalInput").ap()
    ced = nc.dram_tensor("ce", [128, NCALLS * B], dt.float32, kind="ExternalInput").ap()
    wd = nc.dram_tensor("w", [128, 16 * OUT_DIM], dt.bfloat16, kind="ExternalInput").ap()
    bmd = nc.dram_tensor("bm", [128, OUT_DIM], dt.float32, kind="ExternalInput").ap()
    iotad = nc.dram_tensor("iota", [128, 128], dt.bfloat16, kind="ExternalInput").ap()
    identd = nc.dram_tensor("identity", [128, 128], dt.bfloat16, kind="ExternalInput").ap()
    outd = nc.dram_tensor("out", [ROWS_PER_CORE, OUT_DIM], dt.float32, kind="ExternalOutput").ap()

    with tile.TileContext(nc) as tc:
        with (
            tc.tile_pool(name="const", bufs=1) as cpool,
            tc.tile_pool(name="g", bufs=3) as gpool,
            tc.tile_pool(name="s", bufs=3) as spool,
            tc.tile_pool(name="hsb", bufs=3) as hsb_pool,
            tc.tile_pool(name="htsb", bufs=3) as htsb_pool,
            tc.tile_pool(name="osb", bufs=2) as osb_pool,
            tc.tile_pool(name="hps", bufs=2, space="PSUM") as hps_pool,
            tc.tile_pool(name="htps", bufs=2, space="PSUM") as htps_pool,
            tc.tile_pool(name="ops", bufs=2, space="PSUM") as ops_pool,
        ):
            idx_sb = cpool.tile([128, NCALLS * B * 8], dt.int16)
            nc.sync.dma_start(idx_sb[:], idxd[:])
            dstl_sb = cpool.tile([128, NCALLS * B], dt.float32)
            nc.sync.dma_start(dstl_sb[:], dstld[:])
            ce_sb = cpool.tile([128, NCALLS * B], dt.float32)
            nc.sync.dma_start(ce_sb[:], ced[:])
            w_sb = cpool.tile([128, 16 * OUT_DIM], dt.bfloat16)
            nc.sync.dma_start(w_sb[:], wd[:])
            bm_sb = cpool.tile([128, OUT_DIM], dt.float32)
            nc.sync.dma_start(bm_sb[:], bmd[:])
            iota_sb = cpool.tile([128, 128], dt.bfloat16)
            nc.sync.dma_start(iota_sb[:], iotad[:])
            ident = cpool.tile([128, 128], dt.bfloat16)
            nc.sync.dma_start(ident[:], identd[:])

            for t in range(NTILES):
                out_ps = ops_pool.tile([128, OUT_DIM], dt.float32)
                for p in range(N_PATHS):
                    call = t * N_PATHS + p
                    g = gpool.tile([128, B, IN_DIM], dt.bfloat16)
                    nc.gpsimd.dma_gather(
                        g[:],
                        featd[:],
                        idx_sb[:, call * B * 8 : (call + 1) * B * 8],
                        B * 128,
                        B * 128,
                        IN_DIM,
                        single_packet=False,
                    )
                    S = spool.tile([128, B * 128], dt.bfloat16)
                    for bb in range(B):
                        col = call * B + bb
                        nc.vector.tensor_scalar(
                            S[:, bb * 128 : (bb + 1) * 128],
                            iota_sb[:],
                            dstl_sb[:, col : col + 1],
                            ce_sb[:, col : col + 1],
                            op0=mybir.AluOpType.is_equal,
                            op1=mybir.AluOpType.mult,
                        )
                    hp = hps_pool.tile([128, IN_DIM], dt.float32)
                    for bb in range(B):
                        nc.tensor.matmul(
                            hp[:],
                            S[:, bb * 128 : (bb + 1) * 128],
                            g[:, bb, :],
                            start=(bb == 0),
                            stop=(bb == B - 1),
                        )
                    hs = hsb_pool.tile([128, IN_DIM], dt.bfloat16)
                    nc.scalar.copy(hs[:], hp[:])
                    htp = htps_pool.tile([128, IN_DIM], dt.bfloat16)
                    for cc in range(4):
                        nc.tensor.transpose(
                            htp[:, cc * 128 : (cc + 1) * 128],
                            hs[:, cc * 128 : (cc + 1) * 128],
                            ident[:],
                        )
                    hts = htsb_pool.tile([128, IN_DIM], dt.bfloat16)
                    nc.vector.tensor_copy(hts[:], htp[:])
                    for cc in range(4):
                        nc.tensor.matmul(
                            out_ps[:],
                            hts[:, cc * 128 : (cc + 1) * 128],
                            w_sb[:, (p * 4 + cc) * OUT_DIM : (p * 4 + cc + 1) * OUT_DIM],
                            start=(p == 0 and cc == 0),
                            stop=(p == N_PATHS - 1 and cc == 3),
                        )
                os_ = osb_pool.tile([128, OUT_DIM], dt.float32)
                nc.vector.tensor_add(os_[:], out_ps[:], bm_sb[:])
                rows = min(128, ROWS_PER_CORE - t * 128)
                nc.sync.dma_start(outd[t * 128 : t * 128 + rows, :], os_[:rows, :])

    nc.compile()
    _program_cache[B] = nc
    return nc


def _prep_host(feat, src, dst, W, b):
    """Host-side bucketing/padding. Returns (B, shared dict, per-core dicts)."""
    src = np.asarray(src).astype(np.int64)
    dst = np.asarray(dst).astype(np.int64)
    feat = np.asarray(feat, dtype=np.float32)
    W = np.asarray(W, dtype=np.float32)
    b = np.asarray(b, dtype=np.float32)

    feat_bf = feat.astype(BF16)

    # weights laid out [fi_local(128), p*4+chunk, fo] for direct SBUF residence
    Wt = np.empty((128, 16, OUT_DIM), dtype=BF16)
    for p in range(N_PATHS):
        for c in range(4):
            Wt[:, p * 4 + c, :] = W[p, c * 128 : (c + 1) * 128, :].astype(BF16)
    Wt = np.ascontiguousarray(Wt.reshape(128, 16 * OUT_DIM))

    bmean = b.mean(0).astype(np.float32)
    bm_bcast = np.ascontiguousarray(np.broadcast_to(bmean, (128, OUT_DIM)))

    iota_bf = np.ascontiguousarray(
        np.broadcast_to(np.arange(128, dtype=np.float32).astype(BF16), (128, 128))
    )

    # per-edge combined scale: deg_in(dst)^-1/2 * deg_out(src)^-1/2 * 1/4
    sorted_data = []  # per path: (src_sorted, dstl_unused, ce_sorted, dst_sorted)
    for p in range(N_PATHS):
        s, d = src[p], dst[p]
        deg_out = np.maximum(np.bincount(s, minlength=N_NODES), 1).astype(np.float64)
        deg_in = np.maximum(np.bincount(d, minlength=N_NODES), 1).astype(np.float64)
        ce = (deg_in[d] ** -0.5) * (deg_out[s] ** -0.5) * 0.25
        order = np.argsort(d, kind="stable")
        sorted_data.append((s[order], d[order], ce[order]))

    # (core, path, tile) ranges via searchsorted on per-path sorted dst
    bounds = []
    for c in range(NCORES):
        base = c * ROWS_PER_CORE
        for t in range(NTILES):
            lo = base + t * 128
            hi = base + min((t + 1) * 128, ROWS_PER_CORE)
            bounds.append((lo, hi))

    ranges = []  # [path][core*NTILES+t] = (a, b) into sorted arrays
    counts = np.zeros((N_PATHS, NCORES * NTILES), dtype=np.int64)
    for p in range(N_PATHS):
        ds = sorted_data[p][1]
        los = np.array([lo for lo, _ in bounds])
        his = np.array([hi for _, hi in bounds])
        a = np.searchsorted(ds, los, side="left")
        e = np.searchsorted(ds, his, side="left")
        ranges.append((a, e))
        counts[p] = e - a

    B = int(np.ceil(counts.max() / 128))

    per_core = []
    for c in range(NCORES):
        idxw = np.zeros((128, NCALLS * B * 8), dtype=np.int16)
        dstl_cols = np.full((128, NCALLS * B), 200.0, dtype=np.float32)
        ce_cols = np.zeros((128, NCALLS * B), dtype=np.float32)
        for t in range(NTILES):
            lo = c * ROWS_PER_CORE + t * 128
            for p in range(N_PATHS):
                call = t * N_PATHS + p
                a, e = ranges[p][0][c * NTILES + t], ranges[p][1][c * NTILES + t]
                cnt = e - a
                ss = sorted_data[p][0][a:e]
                dd = sorted_data[p][1][a:e] - lo
                cc = sorted_data[p][2][a:e]
                idx_pad = np.zeros(B * 128, dtype=np.int16)
                idx_pad[:cnt] = ss
                dstl_pad = np.full(B * 128, 200.0, dtype=np.float64)
                dstl_pad[:cnt] = dd
                ce_pad = np.zeros(B * 128, dtype=np.float64)
                ce_pad[:cnt] = cc
                # dma_gather wrapped index layout: position j -> [j%16, j//16],
                # replicated across the 8 groups of 16 partitions
                w16 = idx_pad.reshape(B * 8, 16).T  # [16, B*8]
                idxw[:, call * B * 8 : (call + 1) * B * 8] = np.tile(w16, (8, 1))
                dstl_cols[:, call * B : (call + 1) * B] = (
                    dstl_pad.reshape(B, 128).T.astype(np.float32)
                )
                ce_cols[:, call * B : (call + 1) * B] = (
                    ce_pad.reshape(B, 128).T.astype(np.float32)
                )
        per_core.append({"idx": idxw, "dstl": dstl_cols, "ce": ce_cols})

    shared = {
        "feat": feat_bf,
        "w": Wt,
        "bm": bm_bcast,
        "iota": iota_bf,
        "identity": np.eye(128, dtype=BF16),
    }
    return B, shared, per_core


def kernel(feat, src, dst, W, b):
    B, shared, per_core = _prep_host(feat, src, dst, W, b)
    nc = _build_program(B)
    in_maps = [{**shared, **pc} for pc in per_core]
    res = run_bass_kernel_spmd(nc, in_maps, list(range(NCORES)))
    out = np.concatenate([res.results[c]["out"] for c in range(NCORES)], axis=0)
    return out.astype(np.float32)


if __name__ == "__main__":
    rng = np.random.default_rng(0)
    feat = rng.standard_normal((N_NODES, IN_DIM), dtype=np.float32)
    src = rng.integers(0, N_NODES, (N_PATHS, 160000)).astype(np.int64)
    dst = rng.integers(0, N_NODES, (N_PATHS, 160000)).astype(np.int64)
    W = (rng.standard_normal((N_PATHS, IN_DIM, OUT_DIM), dtype=np.float32) / np.sqrt(IN_DIM)).astype(np.float32)
    b = np.zeros((N_PATHS, OUT_DIM), np.float32)
    out = kernel(feat=feat, src=src, dst=dst, W=W, b=b)
    print("kernel ran, out shape", out.shape, out.dtype)



# revision 2
# speedup vs baseline: 1.0183x; 1.0183x over previous
"""MetaPathEncoder as Bass/Tile SPMD kernel on 8 TRN2 cores — v3.

v2 + knobs:
  - per-path dedup of gathered sources (S rows may hold multiple edges)
  - gather calls split into sub-calls of <= SPLIT_BLOCKS*128 indices
  - gathers striped across SWDGE queues (NQUEUES)
"""
import sys

for _p in ("/opt/trn_rl_repo",):
    if _p not in sys.path:
        sys.path.insert(0, _p)

import numpy as np
import ml_dtypes

import concourse.bass as bass
import concourse.tile as tile
from concourse import bacc, mybir
from concourse.bass_utils import run_bass_kernel_spmd

BF16 = ml_dtypes.bfloat16

N_NODES = 10000
N_PATHS = 4
IN_DIM = 512
OUT_DIM = 512
NCORES = 8
ROWS_PER_CORE = N_NODES // NCORES  # 1250
NTILES = (ROWS_PER_CORE + 127) // 128  # 10
NCALLS = NTILES * N_PATHS  # 40

DEDUP = True
SPLIT_BLOCKS = 8  # max 128-blocks per dma_gather sub-call (1024 idx fits the ring)
NQUEUES = 4

_program_cache: dict[tuple, object] = {}


def _build_program(blocks: tuple):
    key = (blocks, SPLIT_BLOCKS, NQUEUES)
    if key in _program_cache:
        return _program_cache[key]

    dt = mybir.dt
    total_blk = sum(blocks)
    nc = bacc.Bacc(
        "TRN2",
        target_bir_lowering=False,
        debug=False,
        num_devices=NCORES,
        num_swdge_queues=NQUEUES,
    )

    featd = nc.dram_tensor("feat", [N_NODES, IN_DIM], dt.bfloat16, kind="ExternalInput").ap()
    idxd = nc.dram_tensor("idx", [128, total_blk * 8], dt.int16, kind="ExternalInput").ap()
    sd = nc.dram_tensor("smat", [128, total_blk * 128], dt.bfloat16, kind="ExternalInput").ap()
    wd = nc.dram_tensor("w", [128, 16 * OUT_DIM], dt.bfloat16, kind="ExternalInput").ap()
    bmd = nc.dram_tensor("bm", [128, OUT_DIM], dt.float32, kind="ExternalInput").ap()
    identd = nc.dram_tensor("identity", [128, 128], dt.bfloat16, kind="ExternalInput").ap()
    outd = nc.dram_tensor("out", [ROWS_PER_CORE, OUT_DIM], dt.float32, kind="ExternalOutput").ap()

    qcounter = [0]

    def next_q():
        q = qcounter[0] % NQUEUES
        qcounter[0] += 1
        return q

    with tile.TileContext(nc) as tc:
        with (
            tc.tile_pool(name="const", bufs=1) as cpool,
            tc.tile_pool(name="g", bufs=4) as gpool,
            tc.tile_pool(name="s", bufs=4) as spool,
            tc.tile_pool(name="hsb", bufs=3) as hsb_pool,
            tc.tile_pool(name="htsb", bufs=3) as htsb_pool,
            tc.tile_pool(name="osb", bufs=2) as osb_pool,
            tc.tile_pool(name="hps", bufs=2, space="PSUM") as hps_pool,
            tc.tile_pool(name="htps", bufs=2, space="PSUM") as htps_pool,
            tc.tile_pool(name="ops", bufs=2, space="PSUM") as ops_pool,
        ):
            idx_sb = cpool.tile([128, total_blk * 8], dt.int16)
            nc.sync.dma_start(idx_sb[:], idxd[:])
            w_sb = cpool.tile([128, 16 * OUT_DIM], dt.bfloat16)
            nc.sync.dma_start(w_sb[:], wd[:])
            bm_sb = cpool.tile([128, OUT_DIM], dt.float32)
            nc.sync.dma_start(bm_sb[:], bmd[:])
            ident = cpool.tile([128, 128], dt.bfloat16)
            nc.sync.dma_start(ident[:], identd[:])

            off = 0
            for t in range(NTILES):
                out_ps = ops_pool.tile([128, OUT_DIM], dt.float32)
                for p in range(N_PATHS):
                    call = t * N_PATHS + p
                    Bc = blocks[call]
                    g = gpool.tile([128, Bc, IN_DIM], dt.bfloat16)
                    # split the gather into sub-calls of <= SPLIT_BLOCKS blocks
                    for j0 in range(0, Bc, SPLIT_BLOCKS):
                        j1 = min(j0 + SPLIT_BLOCKS, Bc)
                        nb = j1 - j0
                        nc.gpsimd.dma_gather(
                            g[:, j0:j1, :],
                            featd[:],
                            idx_sb[:, (off + j0) * 8 : (off + j1) * 8],
                            nb * 128,
                            nb * 128,
                            IN_DIM,
                            single_packet=False,
                            queue_num=next_q(),
                        )
                    S = spool.tile([128, Bc * 128], dt.bfloat16)
                    nc.sync.dma_start(S[:], sd[:, off * 128 : (off + Bc) * 128])
                    hp = hps_pool.tile([128, IN_DIM], dt.float32)
                    for bb in range(Bc):
                        nc.tensor.matmul(
                            hp[:],
                            S[:, bb * 128 : (bb + 1) * 128],
                            g[:, bb, :],
                            start=(bb == 0),
                            stop=(bb == Bc - 1),
                        )
                    hs = hsb_pool.tile([128, IN_DIM], dt.bfloat16)
                    nc.scalar.copy(hs[:], hp[:])
                    htp = htps_pool.tile([128, IN_DIM], dt.bfloat16)
                    for cc in range(4):
                        nc.tensor.transpose(
                            htp[:, cc * 128 : (cc + 1) * 128],
                            hs[:, cc * 128 : (cc + 1) * 128],
                            ident[:],
                        )
                    hts = htsb_pool.tile([128, IN_DIM], dt.bfloat16)
                    nc.vector.tensor_copy(hts[:], htp[:])
                    for cc in range(4):
                        nc.tensor.matmul(
                            out_ps[:],
                            hts[:, cc * 128 : (cc + 1) * 128],
                            w_sb[:, (p * 4 + cc) * OUT_DIM : (p * 4 + cc + 1) * OUT_DIM],
                            start=(p == 0 and cc == 0),
                            stop=(p == N_PATHS - 1 and cc == 3),
                        )
                    off += Bc
                os_ = osb_pool.tile([128, OUT_DIM], dt.float32)
                nc.vector.tensor_add(os_[:], out_ps[:], bm_sb[:])
                rows = min(128, ROWS_PER_CORE - t * 128)
                nc.sync.dma_start(outd[t * 128 : t * 128 + rows, :], os_[:rows, :])

    nc.compile()
    _program_cache[key] = nc
    return nc


def _prep_host(feat, src, dst, W, b):
    src = np.asarray(src).astype(np.int64)
    dst = np.asarray(dst).astype(np.int64)
    feat = np.asarray(feat, dtype=np.float32)
    W = np.asarray(W, dtype=np.float32)
    b = np.asarray(b, dtype=np.float32)

    feat_bf = feat.astype(BF16)

    Wt = np.empty((128, 16, OUT_DIM), dtype=BF16)
    for p in range(N_PATHS):
        for c in range(4):
            Wt[:, p * 4 + c, :] = W[p, c * 128 : (c + 1) * 128, :].astype(BF16)
    Wt = np.ascontiguousarray(Wt.reshape(128, 16 * OUT_DIM))

    bmean = b.mean(0).astype(np.float32)
    bm_bcast = np.ascontiguousarray(np.broadcast_to(bmean, (128, OUT_DIM)))

    sorted_data = []
    for p in range(N_PATHS):
        s, d = src[p], dst[p]
        deg_out = np.maximum(np.bincount(s, minlength=N_NODES), 1).astype(np.float64)
        deg_in = np.maximum(np.bincount(d, minlength=N_NODES), 1).astype(np.float64)
        ce = (deg_in[d] ** -0.5) * (deg_out[s] ** -0.5) * 0.25
        order = np.argsort(d, kind="stable")
        sorted_data.append((s[order], d[order], ce[order]))

    bounds_lo = np.array(
        [c * ROWS_PER_CORE + t * 128 for c in range(NCORES) for t in range(NTILES)]
    )
    bounds_hi = np.array(
        [
            c * ROWS_PER_CORE + min((t + 1) * 128, ROWS_PER_CORE)
            for c in range(NCORES)
            for t in range(NTILES)
        ]
    )

    ranges = []
    for p in range(N_PATHS):
        ds = sorted_data[p][1]
        a = np.searchsorted(ds, bounds_lo, side="left")
        e = np.searchsorted(ds, bounds_hi, side="left")
        ranges.append((a, e))

    # gather per-core bucket data; dedup if enabled; compute slot counts
    # buckets[c][t][p] = (slot_idx_array, edge_slot_pos, dd, cc)
    buckets = [[[None] * N_PATHS for _ in range(NTILES)] for _ in range(NCORES)]
    slot_counts = np.zeros((NCORES, NTILES, N_PATHS), dtype=np.int64)
    for c in range(NCORES):
        for t in range(NTILES):
            lo = c * ROWS_PER_CORE + t * 128
            for p in range(N_PATHS):
                a, e = ranges[p][0][c * NTILES + t], ranges[p][1][c * NTILES + t]
                ss = sorted_data[p][0][a:e]
                dd = (sorted_data[p][1][a:e] - lo).astype(np.int64)
                cc = sorted_data[p][2][a:e]
                if DEDUP:
                    uniq, inv = np.unique(ss, return_inverse=True)
                    buckets[c][t][p] = (uniq, inv, dd, cc)
                    slot_counts[c, t, p] = len(uniq)
                else:
                    pos = np.arange(len(ss))
                    buckets[c][t][p] = (ss, pos, dd, cc)
                    slot_counts[c, t, p] = len(ss)

    blocks = []
    for t in range(NTILES):
        for p in range(N_PATHS):
            mx = slot_counts[:, t, p].max()
            blocks.append(int(np.ceil(mx / 128)) or 1)
    blocks = tuple(blocks)
    total_blk = sum(blocks)

    per_core = []
    for c in range(NCORES):
        idxw = np.zeros((128, total_blk * 8), dtype=np.int16)
        smat = np.zeros((128, total_blk * 128), dtype=np.float32)
        off = 0
        for t in range(NTILES):
            for p in range(N_PATHS):
                call = t * N_PATHS + p
                Bc = blocks[call]
                uniq, inv, dd, cc = buckets[c][t][p]
                idx_pad = np.zeros(Bc * 128, dtype=np.int16)
                idx_pad[: len(uniq)] = uniq
                w16 = idx_pad.reshape(Bc * 8, 16).T
                idxw[:, off * 8 : (off + Bc) * 8] = np.tile(w16, (8, 1))
                # S[slot % 128, off*128 + (slot//128)*128 + dst_local] += c_e
                np.add.at(
                    smat,
                    (inv % 128, off * 128 + (inv // 128) * 128 + dd),
                    cc.astype(np.float32),
                )
                off += Bc
        per_core.append({"idx": idxw, "smat": smat.astype(BF16)})

    shared = {
        "feat": feat_bf,
        "w": Wt,
        "bm": bm_bcast,
        "identity": np.eye(128, dtype=BF16),
    }
    return blocks, shared, per_core


def kernel(feat, src, dst, W, b):
    blocks, shared, per_core = _prep_host(feat, src, dst, W, b)
    nc = _build_program(blocks)
    in_maps = [{**shared, **pc} for pc in per_core]
    res = run_bass_kernel_spmd(nc, in_maps, list(range(NCORES)))
    out = np.concatenate([res.results[c]["out"] for c in range(NCORES)], axis=0)
    return out.astype(np.float32)


if __name__ == "__main__":
    rng = np.random.default_rng(0)
    feat = rng.standard_normal((N_NODES, IN_DIM), dtype=np.float32)
    src = rng.integers(0, N_NODES, (N_PATHS, 160000)).astype(np.int64)
    dst = rng.integers(0, N_NODES, (N_PATHS, 160000)).astype(np.int64)
    W = (rng.standard_normal((N_PATHS, IN_DIM, OUT_DIM), dtype=np.float32) / np.sqrt(IN_DIM)).astype(np.float32)
    b = np.zeros((N_PATHS, OUT_DIM), np.float32)
    out = kernel(feat=feat, src=src, dst=dst, W=W, b=b)
    print("kernel ran, out shape", out.shape, out.dtype)


# revision 3
# speedup vs baseline: 1.0310x; 1.0125x over previous
"""MetaPathEncoder as Bass/Tile SPMD kernel on 8 TRN2 cores — v3.

v2 + knobs:
  - per-path dedup of gathered sources (S rows may hold multiple edges)
  - gather calls split into sub-calls of <= SPLIT_BLOCKS*128 indices
  - gathers striped across SWDGE queues (NQUEUES)
"""
import sys

for _p in ("/opt/trn_rl_repo",):
    if _p not in sys.path:
        sys.path.insert(0, _p)

import numpy as np
import ml_dtypes

import concourse.bass as bass
import concourse.tile as tile
from concourse import bacc, mybir
from concourse.bass_utils import run_bass_kernel_spmd

BF16 = ml_dtypes.bfloat16

N_NODES = 10000
N_PATHS = 4
IN_DIM = 512
OUT_DIM = 512
NCORES = 8
ROWS_PER_CORE = N_NODES // NCORES  # 1250
NTILES = (ROWS_PER_CORE + 127) // 128  # 10
NCALLS = NTILES * N_PATHS  # 40

DEDUP = True
SPLIT_BLOCKS = 8  # max 128-blocks per dma_gather sub-call (1024 idx fits the ring)
NQUEUES = 4

_program_cache: dict[tuple, object] = {}


def _build_program(blocks: tuple):
    key = (blocks, SPLIT_BLOCKS, NQUEUES)
    if key in _program_cache:
        return _program_cache[key]

    dt = mybir.dt
    total_blk = sum(blocks)
    nc = bacc.Bacc(
        "TRN2",
        target_bir_lowering=False,
        debug=False,
        num_devices=NCORES,
        num_swdge_queues=NQUEUES,
    )

    featd = nc.dram_tensor("feat", [N_NODES, IN_DIM], dt.bfloat16, kind="ExternalInput").ap()
    idxd = nc.dram_tensor("idx", [128, total_blk * 8], dt.int16, kind="ExternalInput").ap()
    sd = nc.dram_tensor("smat", [128, total_blk * 128], dt.bfloat16, kind="ExternalInput").ap()
    wd = nc.dram_tensor("w", [128, 16 * OUT_DIM], dt.bfloat16, kind="ExternalInput").ap()
    bmd = nc.dram_tensor("bm", [128, OUT_DIM], dt.float32, kind="ExternalInput").ap()
    identd = nc.dram_tensor("identity", [128, 128], dt.bfloat16, kind="ExternalInput").ap()
    outd = nc.dram_tensor("out", [ROWS_PER_CORE, OUT_DIM], dt.float32, kind="ExternalOutput").ap()

    qcounter = [0]

    def next_q():
        q = qcounter[0] % NQUEUES
        qcounter[0] += 1
        return q

    with tile.TileContext(nc) as tc:
        with (
            tc.tile_pool(name="const", bufs=1) as cpool,
            tc.tile_pool(name="g", bufs=6) as gpool,
            tc.tile_pool(name="s", bufs=6) as spool,
            tc.tile_pool(name="hsb", bufs=4) as hsb_pool,
            tc.tile_pool(name="htsb", bufs=4) as htsb_pool,
            tc.tile_pool(name="osb", bufs=2) as osb_pool,
            tc.tile_pool(name="hps", bufs=2, space="PSUM") as hps_pool,
            tc.tile_pool(name="htps", bufs=2, space="PSUM") as htps_pool,
            tc.tile_pool(name="ops", bufs=2, space="PSUM") as ops_pool,
        ):
            idx_sb = cpool.tile([128, total_blk * 8], dt.int16)
            nc.sync.dma_start(idx_sb[:], idxd[:])
            w_sb = cpool.tile([128, 16 * OUT_DIM], dt.bfloat16)
            nc.sync.dma_start(w_sb[:], wd[:])
            bm_sb = cpool.tile([128, OUT_DIM], dt.float32)
            nc.sync.dma_start(bm_sb[:], bmd[:])
            ident = cpool.tile([128, 128], dt.bfloat16)
            nc.sync.dma_start(ident[:], identd[:])

            off = 0
            for t in range(NTILES):
                out_ps = ops_pool.tile([128, OUT_DIM], dt.float32)
                for p in range(N_PATHS):
                    call = t * N_PATHS + p
                    Bc = blocks[call]
                    g = gpool.tile([128, Bc, IN_DIM], dt.bfloat16)
                    # split the gather into sub-calls of <= SPLIT_BLOCKS blocks
                    for j0 in range(0, Bc, SPLIT_BLOCKS):
                        j1 = min(j0 + SPLIT_BLOCKS, Bc)
                        nb = j1 - j0
                        nc.gpsimd.dma_gather(
                            g[:, j0:j1, :],
                            featd[:],
                            idx_sb[:, (off + j0) * 8 : (off + j1) * 8],
                            nb * 128,
                            nb * 128,
                            IN_DIM,
                            single_packet=False,
                            queue_num=next_q(),
                        )
                    S = spool.tile([128, Bc * 128], dt.bfloat16)
                    nc.sync.dma_start(S[:], sd[:, off * 128 : (off + Bc) * 128])
                    hp = hps_pool.tile([128, IN_DIM], dt.float32)
                    for bb in range(Bc):
                        nc.tensor.matmul(
                            hp[:],
                            S[:, bb * 128 : (bb + 1) * 128],
                            g[:, bb, :],
                            start=(bb == 0),
                            stop=(bb == Bc - 1),
                        )
                    hs = hsb_pool.tile([128, IN_DIM], dt.bfloat16)
                    nc.scalar.copy(hs[:], hp[:])
                    htp = htps_pool.tile([128, IN_DIM], dt.bfloat16)
                    for cc in range(4):
                        nc.tensor.transpose(
                            htp[:, cc * 128 : (cc + 1) * 128],
                            hs[:, cc * 128 : (cc + 1) * 128],
                            ident[:],
                        )
                    hts = htsb_pool.tile([128, IN_DIM], dt.bfloat16)
                    nc.vector.tensor_copy(hts[:], htp[:])
                    for cc in range(4):
                        nc.tensor.matmul(
                            out_ps[:],
                            hts[:, cc * 128 : (cc + 1) * 128],
                            w_sb[:, (p * 4 + cc) * OUT_DIM : (p * 4 + cc + 1) * OUT_DIM],
                            start=(p == 0 and cc == 0),
                            stop=(p == N_PATHS - 1 and cc == 3),
                        )
                    off += Bc
                os_ = osb_pool.tile([128, OUT_DIM], dt.float32)
                nc.vector.tensor_add(os_[:], out_ps[:], bm_sb[:])
                rows = min(128, ROWS_PER_CORE - t * 128)
                nc.sync.dma_start(outd[t * 128 : t * 128 + rows, :], os_[:rows, :])

    nc.compile()
    _program_cache[key] = nc
    return nc


def _prep_host(feat, src, dst, W, b):
    src = np.asarray(src).astype(np.int64)
    dst = np.asarray(dst).astype(np.int64)
    feat = np.asarray(feat, dtype=np.float32)
    W = np.asarray(W, dtype=np.float32)
    b = np.asarray(b, dtype=np.float32)

    feat_bf = feat.astype(BF16)

    Wt = np.empty((128, 16, OUT_DIM), dtype=BF16)
    for p in range(N_PATHS):
        for c in range(4):
            Wt[:, p * 4 + c, :] = W[p, c * 128 : (c + 1) * 128, :].astype(BF16)
    Wt = np.ascontiguousarray(Wt.reshape(128, 16 * OUT_DIM))

    bmean = b.mean(0).astype(np.float32)
    bm_bcast = np.ascontiguousarray(np.broadcast_to(bmean, (128, OUT_DIM)))

    sorted_data = []
    for p in range(N_PATHS):
        s, d = src[p], dst[p]
        deg_out = np.maximum(np.bincount(s, minlength=N_NODES), 1).astype(np.float64)
        deg_in = np.maximum(np.bincount(d, minlength=N_NODES), 1).astype(np.float64)
        ce = (deg_in[d] ** -0.5) * (deg_out[s] ** -0.5) * 0.25
        order = np.argsort(d, kind="stable")
        sorted_data.append((s[order], d[order], ce[order]))

    bounds_lo = np.array(
        [c * ROWS_PER_CORE + t * 128 for c in range(NCORES) for t in range(NTILES)]
    )
    bounds_hi = np.array(
        [
            c * ROWS_PER_CORE + min((t + 1) * 128, ROWS_PER_CORE)
            for c in range(NCORES)
            for t in range(NTILES)
        ]
    )

    ranges = []
    for p in range(N_PATHS):
        ds = sorted_data[p][1]
        a = np.searchsorted(ds, bounds_lo, side="left")
        e = np.searchsorted(ds, bounds_hi, side="left")
        ranges.append((a, e))

    # gather per-core bucket data; dedup if enabled; compute slot counts
    # buckets[c][t][p] = (slot_idx_array, edge_slot_pos, dd, cc)
    buckets = [[[None] * N_PATHS for _ in range(NTILES)] for _ in range(NCORES)]
    slot_counts = np.zeros((NCORES, NTILES, N_PATHS), dtype=np.int64)
    for c in range(NCORES):
        for t in range(NTILES):
            lo = c * ROWS_PER_CORE + t * 128
            for p in range(N_PATHS):
                a, e = ranges[p][0][c * NTILES + t], ranges[p][1][c * NTILES + t]
                ss = sorted_data[p][0][a:e]
                dd = (sorted_data[p][1][a:e] - lo).astype(np.int64)
                cc = sorted_data[p][2][a:e]
                if DEDUP:
                    uniq, inv = np.unique(ss, return_inverse=True)
                    buckets[c][t][p] = (uniq, inv, dd, cc)
                    slot_counts[c, t, p] = len(uniq)
                else:
                    pos = np.arange(len(ss))
                    buckets[c][t][p] = (ss, pos, dd, cc)
                    slot_counts[c, t, p] = len(ss)

    blocks = []
    for t in range(NTILES):
        for p in range(N_PATHS):
            mx = slot_counts[:, t, p].max()
            blocks.append(int(np.ceil(mx / 128)) or 1)
    blocks = tuple(blocks)
    total_blk = sum(blocks)

    per_core = []
    for c in range(NCORES):
        idxw = np.zeros((128, total_blk * 8), dtype=np.int16)
        smat = np.zeros((128, total_blk * 128), dtype=np.float32)
        off = 0
        for t in range(NTILES):
            for p in range(N_PATHS):
                call = t * N_PATHS + p
                Bc = blocks[call]
                uniq, inv, dd, cc = buckets[c][t][p]
                idx_pad = np.zeros(Bc * 128, dtype=np.int16)
                idx_pad[: len(uniq)] = uniq
                w16 = idx_pad.reshape(Bc * 8, 16).T
                idxw[:, off * 8 : (off + Bc) * 8] = np.tile(w16, (8, 1))
                # S[slot % 128, off*128 + (slot//128)*128 + dst_local] += c_e
                np.add.at(
                    smat,
                    (inv % 128, off * 128 + (inv // 128) * 128 + dd),
                    cc.astype(np.float32),
                )
                off += Bc
        per_core.append({"idx": idxw, "smat": smat.astype(BF16)})

    shared = {
        "feat": feat_bf,
        "w": Wt,
        "bm": bm_bcast,
        "identity": np.eye(128, dtype=BF16),
    }
    return blocks, shared, per_core


def kernel(feat, src, dst, W, b):
    blocks, shared, per_core = _prep_host(feat, src, dst, W, b)
    nc = _build_program(blocks)
    in_maps = [{**shared, **pc} for pc in per_core]
    res = run_bass_kernel_spmd(nc, in_maps, list(range(NCORES)))
    out = np.concatenate([res.results[c]["out"] for c in range(NCORES)], axis=0)
    return out.astype(np.float32)


if __name__ == "__main__":
    rng = np.random.default_rng(0)
    feat = rng.standard_normal((N_NODES, IN_DIM), dtype=np.float32)
    src = rng.integers(0, N_NODES, (N_PATHS, 160000)).astype(np.int64)
    dst = rng.integers(0, N_NODES, (N_PATHS, 160000)).astype(np.int64)
    W = (rng.standard_normal((N_PATHS, IN_DIM, OUT_DIM), dtype=np.float32) / np.sqrt(IN_DIM)).astype(np.float32)
    b = np.zeros((N_PATHS, OUT_DIM), np.float32)
    out = kernel(feat=feat, src=src, dst=dst, W=W, b=b)
    print("kernel ran, out shape", out.shape, out.dtype)
